# revision 33
# baseline (speedup 1.0000x reference)
"""Bahdanau (additive) attention Trainium2 kernel — factorized-score version.

Full-input contract: kernel(**inputs) takes the unsharded inputs
(query [16,128,256], value [16,256,256], mask [16,256], W1 [256,256],
W2 [256,256], scale [256]) and returns (context, attn_weights), both
[16,128,256] float32, matching the jax reference.

Sharding: data-parallel over batch -> 8 NeuronCores x 2 batches each.

Algorithm (replaces elementwise tanh over t*s*u = 16.8M elems/core):
  tanh(q+k) ~ g(q) + sum_r A_r F_r(q) G_r(k) with sinusoid factors at
  log-spaced frequencies {0.16, 0.28, 0.56, 1.12, 2.24}:
    slots 0-4: sin/cos at 0.16, 0.28 and sin at 0.56 — direct ScalarE Sin
               (args stay inside the +-3.3 rad domain of the HW table)
    slots 5-9: cos 0.56, sin/cos 1.12, sin/cos 2.24 via fp16 doubling
               identities (c2m = 1-2 s_m^2, s2m = 2 s_m c_m); squares on
               GpSimd (q side) / ScalarE Square (k side), rest on DVE
  g(q) is dropped (row-constant cancels in softmax). 9 product ranks +
  4 k-only ranks + the mask row fold into PE matmuls accumulating
  scores[t,s] in PSUM; A_r*scale_u folds into one fp16 multiply per
  u-block on the q-side stack (broadcast table, stride-0 AP).
  softmax: Exp on ScalarE with fused row sums; attn out via Copy(scale).
  context = attn @ value with PE transposes of e (scaled 2^-6, fp16).

Fit (vs f64 reference, incl. fp16 emulation): rel err ctx 9.6e-3,
attn 1.03e-2 (tolerance 2e-2).
"""

import sys

if "/opt/trn_rl_repo" not in sys.path:
    sys.path.insert(0, "/opt/trn_rl_repo")

from contextlib import ExitStack

import numpy as np

import concourse.bacc as bacc
import concourse.bass as bass
import concourse.tile as tile
from concourse import mybir
from concourse.bass_utils import run_bass_kernel_spmd

F32 = mybir.dt.float32
F16 = mybir.dt.float16
AF = mybir.ActivationFunctionType
ALU = mybir.AluOpType

N_CORES = 8
B = 2          # batches per core
T = 128        # query rows
S = 256        # kv rows
D = 256        # d_model
U = 256        # units
NSLOT = 10

F1 = 0.16
F2 = 0.28

# slots: 0:s(f1) 1:c(f1) 2:s(f2) 3:c(f2) 4:s(2f2)
#        5:c(2f2) 6:s(4f2) 7:c(4f2) 8:s(8f2) 9:c(8f2)
# (q-slot, k-slot, A); q-slots injective so A_r*scale_u folds per q-slot
RANKS = [
    (0, 1, 9.075957),
    (1, 2, 24.655758),
    (5, 4, -1.303213),
    (7, 6, 0.427516),
    (6, 7, 0.344912),
    (2, 7, -0.017192),
    (9, 8, 0.07271),
    (3, 8, -0.020565),
    (8, 9, 0.07767),
]
KONLY = [(0, -14.361238), (2, -12.338257), (4, -0.182925), (8, 0.019671)]

AMPQ = np.zeros(NSLOT, dtype=np.float32)
for _qs, _ks, _a in RANKS:
    AMPQ[_qs] = _a


def build_bass() -> bass.Bass:
    nc = bacc.Bacc("TRN2", target_bir_lowering=False, debug=False)

    # fused fp16 input blobs (vS is derived on-device from vT transposes)
    # blobA: [w1(512) | qT(512)]   blobB: [w2(512) | vT(1024) | ampsc | scN]
    BLOBA = 512 + 512 + 128
    BLOBB = 512 + 1024 + NSLOT * 2 + 2 * len(KONLY)
    blobA_in = nc.dram_tensor("blobA", [128, BLOBA], F16, kind="ExternalInput")
    blobB_in = nc.dram_tensor("blobB", [128, BLOBB], F16, kind="ExternalInput")
    mrow_in = nc.dram_tensor("mrow", [1, B, S], F32, kind="ExternalInput")
    ctx_out = nc.dram_tensor("context", [B, T, D], F32, kind="ExternalOutput")
    attn_out = nc.dram_tensor("attn", [B, T, S], F32, kind="ExternalOutput")


    with tile.TileContext(nc) as tc, ExitStack() as ctx:
        sg = ctx.enter_context(tc.tile_pool(name="sg", bufs=1))
        p_qu = ctx.enter_context(tc.tile_pool(name="p_qu", bufs=1, space="PSUM"))
        p_ku = ctx.enter_context(tc.tile_pool(name="p_ku", bufs=1, space="PSUM"))
        p_sc = ctx.enter_context(tc.tile_pool(name="p_sc", bufs=1, space="PSUM"))
        p_rc = ctx.enter_context(tc.tile_pool(name="p_rc", bufs=1, space="PSUM"))
        p_ct = ctx.enter_context(tc.tile_pool(name="p_ct", bufs=1, space="PSUM"))
        p_tp = ctx.enter_context(tc.tile_pool(name="p_tp", bufs=1, space="PSUM"))

        # ---- input DMAs (two blobs ordered by need + tiny ones)
        blobA = sg.tile([128, BLOBA], F16)
        nc.sync.dma_start(out=blobA, in_=blobA_in[:, :])
        blobB = sg.tile([128, BLOBB], F16)
        nc.gpsimd.dma_start(out=blobB, in_=blobB_in[:, :])
        w1 = blobA[:, 0:512].rearrange("p (j u) -> p j u", j=2)
        qT = blobA[:, 512:1024].rearrange("p (j b t) -> p j b t", j=2, b=B)
        w2 = blobB[:, 0:512].rearrange("p (j u) -> p j u", j=2)
        vT = blobB[:, 512:1536].rearrange("p (j b s) -> p j b s", j=2, b=B)
        ampsc = blobB[:, 1536:1536 + NSLOT * 2].rearrange(
            "p (f u) -> p f u", f=NSLOT)
        scN = blobB[:, 1536 + NSLOT * 2:BLOBB].rearrange(
            "p (u k) -> p u k", u=2)
        id16 = blobA[:, 1024:1152]
        ones16 = sg.tile([1, 128], F16)
        nc.vector.memset(ones16, 1.0)
        mrow = sg.tile([1, B, S], F32)
        nc.sync.dma_start(out=mrow, in_=mrow_in[:, :, :])

        sc_b = [p_sc.tile([128, S], F32, tag=f"scores{b}", name=f"sc{b}")
                for b in range(B)]

        # PE clock warm-up during input DMA
        wjunk = sg.tile([128, 512], F16)
        nc.vector.memset(wjunk, 0.0)
        for w in range(4):
            nc.tensor.matmul(
                sc_b[w % B],
                lhsT=wjunk[:, 0:128], rhs=wjunk[:, 0:256],
                start=True, stop=True,
            )

        pibias = sg.tile([128, 1], F32)
        nc.vector.memset(pibias, np.pi / 2)
        obias = sg.tile([128, 1], F32)
        nc.vector.memset(obias, 1.0)

        # ---- preamble: qU[u,(b,t)] = W1^T q^T ; kU[u,(b,s)] = W2^T v^T
        qU = p_qu.tile([128, B, 2, T], F32, tag="qU")
        for b in range(B):
            for ub in range(2):
                for j in range(2):
                    nc.tensor.matmul(
                        qU[:, b, ub, :],
                        lhsT=w1[:, j, ub * 128:(ub + 1) * 128],
                        rhs=qT[:, j, b, :],
                        start=(j == 0), stop=(j == 1),
                    )
        kU = p_ku.tile([128, B, 2, S], F32, tag="kU")
        for b in range(B):
            for ub in range(2):
                for j in range(2):
                    nc.tensor.matmul(
                        kU[:, b, ub, :],
                        lhsT=w2[:, j, ub * 128:(ub + 1) * 128],
                        rhs=vT[:, j, b, :],
                        start=(j == 0), stop=(j == 1),
                    )

        # vS[s-part, sblk, b, d] from vT via PE transposes; copies on GpSimd
        vS = sg.tile([128, 2, B, D], F16)
        for b in range(B):
            for sb in range(2):
                for j in range(2):
                    tp = p_tp.tile([128, 128], F16, tag="tp", name=f"tpv{b}{sb}{j}")
                    nc.tensor.transpose(
                        tp, vT[:, j, b, sb * 128:(sb + 1) * 128], id16)
                    nc.vector.tensor_copy(
                        out=vS[:, sb, b, j * 128:(j + 1) * 128], in_=tp)

        qstack = sg.tile([128, NSLOT, B, 2, T], F16)
        kstack = sg.tile([128, NSLOT, B, 2, S], F16)

        # base sinusoids on ScalarE (q side first so its ladder starts early)
        def sins(stack, src):
            nc.scalar.activation(out=stack[:, 0], in_=src, func=AF.Sin,
                                 scale=F1)
            nc.scalar.activation(out=stack[:, 1], in_=src, func=AF.Sin,
                                 scale=F1, bias=pibias)
            nc.scalar.activation(out=stack[:, 2], in_=src, func=AF.Sin,
                                 scale=F2)
            nc.scalar.activation(out=stack[:, 3], in_=src, func=AF.Sin,
                                 scale=F2, bias=pibias)
            nc.scalar.activation(out=stack[:, 4], in_=src, func=AF.Sin,
                                 scale=2 * F2)

        sins(kstack, kU[:, :, :, :])
        sins(qstack, qU[:, :, :, :])

        # doubling ladder: (sq_src, c_dst) then s_dst = 2 * s_src * c_dst
        def ladder(stack, scrtag, width, sq_engines, c_act=False):
            for (src, cdst, sprev, sdst), eng in zip(
                    ((2, 5, 4, 6), (4, 7, 6, 8), (6, 9, None, None)),
                    sq_engines):
                scr = sg.tile([128, B, 2, width], F16, tag=f"{scrtag}{src}")
                if eng == "act":
                    nc.scalar.activation(out=scr, in_=stack[:, src],
                                         func=AF.Square)
                elif eng == "gp":
                    nc.gpsimd.tensor_tensor(out=scr, in0=stack[:, src],
                                            in1=stack[:, src], op=ALU.mult)
                else:
                    nc.vector.tensor_tensor(out=scr, in0=stack[:, src],
                                            in1=stack[:, src], op=ALU.mult)
                if c_act:
                    # c = -2*sq + 1 on the otherwise-idle ScalarE (Copy is
                    # in every activation table: no table switch)
                    nc.scalar.activation(out=stack[:, cdst], in_=scr,
                                         func=AF.Copy, scale=-2.0, bias=1.0)
                else:
                    nc.vector.tensor_scalar(out=stack[:, cdst], in0=scr,
                                            scalar1=-2.0, scalar2=1.0,
                                            op0=ALU.mult, op1=ALU.add)
                if sdst is not None:
                    nc.vector.scalar_tensor_tensor(
                        out=stack[:, sdst], in0=stack[:, sprev], scalar=2.0,
                        in1=stack[:, cdst], op0=ALU.mult, op1=ALU.mult)

        ladder(qstack, "sq", T, ("gp", "gp", "gp"))

        # fold A_r * scale_u into the q-side stack (fp16, per u-block) —
        # emitted before the k ladder so PE rank matmuls can start on the
        # Act-direct k slots while DVE finishes the k ladder
        qsc = sg.tile([128, NSLOT, B, 2, T], F16)

        def fold(ub, lo, hi):
            col = ampsc[:, lo:hi, ub]
            amp_ap = bass.AP(
                tensor=col.tensor, offset=col.offset,
                ap=[list(col.ap[0]), list(col.ap[1]), [0, B], [0, T]],
            )
            nc.vector.tensor_tensor(
                out=qsc[:, lo:hi, :, ub, :], in0=qstack[:, lo:hi, :, ub, :],
                in1=amp_ap, op=ALU.mult)

        # direct slots fold first (unblocks the direct-k rank matmuls),
        # ladder slots fold after the k-ladder's DVE ops
        fold(0, 0, 5)
        fold(1, 0, 5)
        ladder(kstack, "sk", S, ("act", "act", "dve"), c_act=True)
        fold(0, 5, NSLOT)
        fold(1, 5, NSLOT)
        # dummy exp hoists the Exp table load into the matmul phase; reads
        # a k-ladder output so the scheduler keeps it after the Sin/Square ops
        dummy = sg.tile([128, 1], F32)
        nc.scalar.activation(out=dummy, in_=kstack[:, 9, 0, 0, 0:1],
                             func=AF.Exp)

        # ---- k-only ranks -> bias row (PSUM row 0), + mask row
        rows = p_rc.tile([128, B, S], F32, tag="rows")
        for b in range(B):
            i = 0
            n = 2 * len(KONLY)
            for ki, (ks, _a) in enumerate(KONLY):
                for ub in range(2):
                    nc.tensor.matmul(
                        rows[0:1, b, :],
                        lhsT=scN[:, ub, ki:ki + 1],
                        rhs=kstack[:, ks, b, ub, :],
                        start=(i == 0), stop=(i == n - 1),
                    )
                    i += 1
        brow0 = sg.tile([1, B, S], F32)
        brow = sg.tile([1, B, S], F16)

        # ---- per-batch pipeline: scores -> softmax -> context, so batch 1
        # matmuls overlap batch 0 softmax/epilogue
        ORDER = [(0, 1), (1, 2), (5, 4), (7, 6), (6, 7), (2, 7), (9, 8),
                 (3, 8), (8, 9)]
        e = sg.tile([128, B, S], F32)
        esum = sg.tile([128, B, 1], F32)
        inv = sg.tile([128, B, 1], F32)
        attn_f = sg.tile([128, B, S], F32)
        e16 = sg.tile([128, B, S], F16)
        attnT = sg.tile([128, 2, B, T], F16)
        ctxp = p_ct.tile([128, B, D], F32, tag="ctxp")
        ctx_f = sg.tile([128, B, D], F32)
        for b in range(B):
            nc.scalar.activation(out=brow0[:, b, :], in_=rows[0:1, b, :],
                                 func=AF.Copy)
            nc.gpsimd.tensor_tensor(out=brow[:, b, :], in0=brow0[:, b, :],
                                    in1=mrow[:, b, :], op=ALU.add)
            i = 0
            for ub in range(2):
                for (qs, ks) in ORDER:
                    nc.tensor.matmul(
                        sc_b[b],
                        lhsT=qsc[:, qs, b, ub, :],
                        rhs=kstack[:, ks, b, ub, :],
                        start=(i == 0), stop=False,
                    )
                    i += 1
            nc.tensor.matmul(
                sc_b[b], lhsT=ones16, rhs=brow[0:1, b, :],
                start=False, stop=True,
            )
            # softmax for this batch (overlaps next batch's matmuls)
            nc.scalar.activation(out=e[:, b, :], in_=sc_b[b],
                                 func=AF.Exp, accum_out=esum[:, b, :])
            nc.vector.reciprocal(out=inv[:, b, :], in_=esum[:, b, :])
            nc.vector.tensor_scalar_mul(out=attn_f[:, b, :], in0=e[:, b, :],
                                        scalar1=inv[:, b, :])
            nc.sync.dma_start(out=attn_out[b], in_=attn_f[:, b, :])
            nc.vector.tensor_scalar_mul(out=e16[:, b, :], in0=e[:, b, :],
                                        scalar1=2.0 ** -6)
            for sb in range(2):
                tp = p_tp.tile([128, 128], F16, tag="tp")
                nc.tensor.transpose(tp, e16[:, b, sb * 128:(sb + 1) * 128],
                                    id16)
                nc.vector.tensor_copy(out=attnT[:, sb, b, :], in_=tp)
            for sb in range(2):
                nc.tensor.matmul(
                    ctxp[:, b, :], lhsT=attnT[:, sb, b, :],
                    rhs=vS[:, sb, b, :],
                    start=(sb == 0), stop=(sb == 1),
                )
            # ctx = ctxp * inv * 64 in one tensor_scalar (two scalar ops)
            nc.vector.tensor_scalar(out=ctx_f[:, b, :], in0=ctxp[:, b, :],
                                    scalar1=inv[:, b, :], scalar2=64.0,
                                    op0=ALU.mult, op1=ALU.mult)
            nc.sync.dma_start(out=ctx_out[b], in_=ctx_f[:, b, :])

    nc.compile()
    return nc


_BUILT: bass.Bass | None = None


def _get_built() -> bass.Bass:
    global _BUILT
    if _BUILT is None:
        _BUILT = build_bass()
    return _BUILT


def make_in_maps(query, value, mask, W1, W2, scale):
    q16 = np.asarray(query, dtype=np.float16)
    v16 = np.asarray(value, dtype=np.float16)
    m = np.asarray(mask).astype(np.float32)
    w1 = np.asarray(W1, dtype=np.float16)
    w2 = np.asarray(W2, dtype=np.float16)
    sc = np.asarray(scale, dtype=np.float32)

    w1h = np.ascontiguousarray(w1.reshape(2, 128, U).transpose(1, 0, 2))
    w2h = np.ascontiguousarray(w2.reshape(2, 128, U).transpose(1, 0, 2))
    scT = sc.reshape(2, 128).T                       # (128, 2) by u-block
    ampsc = np.ascontiguousarray(
        (AMPQ[None, :, None] * scT[:, None, :]).astype(np.float16))
    scn = np.stack([a * sc for (_ks, a) in KONLY], axis=1)  # (256, nk)
    scN = np.ascontiguousarray(
        scn.reshape(2, 128, len(KONLY)).transpose(1, 0, 2).astype(np.float16))

    in_maps = []
    for c in range(N_CORES):
        sl = slice(B * c, B * (c + 1))
        q = q16[sl]                      # (B, T, D)
        v = v16[sl]                      # (B, S, D)
        qTh = np.ascontiguousarray(
            q.reshape(B, T, 2, 128).transpose(3, 2, 0, 1))
        vTh = np.ascontiguousarray(
            v.reshape(B, S, 2, 128).transpose(3, 2, 0, 1))
        mrow = np.ascontiguousarray(
            ((m[sl] - 1.0) * 30000.0)[None, :, :].astype(np.float32))
        blobA = np.ascontiguousarray(np.concatenate(
            [a.reshape(128, -1) for a in
             (w1h, qTh, np.eye(128, dtype=np.float16))], axis=1))
        blobB = np.ascontiguousarray(np.concatenate(
            [a.reshape(128, -1) for a in (w2h, vTh, ampsc, scN)], axis=1))
        in_maps.append({"blobA": blobA, "blobB": blobB, "mrow": mrow})
    return in_maps


def run(query, value, mask, W1, W2, scale, trace=False, **trace_kwargs):
    nc = _get_built()
    in_maps = make_in_maps(query, value, mask, W1, W2, scale)
    res = run_bass_kernel_spmd(
        nc, in_maps, core_ids=list(range(N_CORES)), trace=trace, **trace_kwargs
    )
    context = np.concatenate([r["context"] for r in res.results], axis=0)
    attn = np.concatenate([r["attn"] for r in res.results], axis=0)
    return (context, attn), res


def kernel(query, value, mask, W1, W2, scale):
    (context, attn), _ = run(query, value, mask, W1, W2, scale, trace=False)
    return context, attn


if __name__ == "__main__":
    build_bass()
    print("build OK")


# revision 35
# speedup vs baseline: 1.0044x; 1.0044x over previous
"""Bahdanau (additive) attention Trainium2 kernel — factorized-score version.

Full-input contract: kernel(**inputs) takes the unsharded inputs
(query [16,128,256], value [16,256,256], mask [16,256], W1 [256,256],
W2 [256,256], scale [256]) and returns (context, attn_weights), both
[16,128,256] float32, matching the jax reference.

Sharding: data-parallel over batch -> 8 NeuronCores x 2 batches each.

Algorithm (replaces elementwise tanh over t*s*u = 16.8M elems/core):
  tanh(q+k) ~ g(q) + sum_r A_r F_r(q) G_r(k) with sinusoid factors at
  log-spaced frequencies {0.16, 0.28, 0.56, 1.12, 2.24}:
    slots 0-4: sin/cos at 0.16, 0.28 and sin at 0.56 — direct ScalarE Sin
               (args stay inside the +-3.3 rad domain of the HW table)
    slots 5-9: cos 0.56, sin/cos 1.12, sin/cos 2.24 via fp16 doubling
               identities (c2m = 1-2 s_m^2, s2m = 2 s_m c_m); squares on
               GpSimd (q side) / ScalarE Square (k side), rest on DVE
  g(q) is dropped (row-constant cancels in softmax). 9 product ranks +
  4 k-only ranks + the mask row fold into PE matmuls accumulating
  scores[t,s] in PSUM; A_r*scale_u folds into one fp16 multiply per
  u-block on the q-side stack (broadcast table, stride-0 AP).
  softmax: Exp on ScalarE with fused row sums; attn out via Copy(scale).
  context = attn @ value with PE transposes of e (scaled 2^-6, fp16).

Fit (vs f64 reference, incl. fp16 emulation): rel err ctx 9.6e-3,
attn 1.03e-2 (tolerance 2e-2).
"""

import sys

if "/opt/trn_rl_repo" not in sys.path:
    sys.path.insert(0, "/opt/trn_rl_repo")

from contextlib import ExitStack

import numpy as np

import concourse.bacc as bacc
import concourse.bass as bass
import concourse.tile as tile
from concourse import mybir
from concourse.bass_utils import run_bass_kernel_spmd

F32 = mybir.dt.float32
F16 = mybir.dt.float16
AF = mybir.ActivationFunctionType
ALU = mybir.AluOpType

N_CORES = 8
B = 2          # batches per core
T = 128        # query rows
S = 256        # kv rows
D = 256        # d_model
U = 256        # units
NSLOT = 10

F1 = 0.16
F2 = 0.28

# slots: 0:s(f1) 1:c(f1) 2:s(f2) 3:c(f2) 4:s(2f2)
#        5:c(2f2) 6:s(4f2) 7:c(4f2) 8:s(8f2) 9:c(8f2)
# (q-slot, k-slot, A); q-slots injective so A_r*scale_u folds per q-slot
RANKS = [
    (0, 1, 9.076809),
    (1, 2, 23.773289),
    (5, 4, -1.240546),
    (7, 6, 0.421618),
    (6, 7, 0.344957),
    (2, 7, -0.017345),
    (9, 8, 0.072694),
    (8, 9, 0.077664),
]
KONLY = [(0, -13.445086), (2, -12.153659), (4, -0.148668)]

AMPQ = np.zeros(NSLOT, dtype=np.float32)
for _qs, _ks, _a in RANKS:
    AMPQ[_qs] = _a


def build_bass() -> bass.Bass:
    nc = bacc.Bacc("TRN2", target_bir_lowering=False, debug=False)

    # fused fp16 input blobs (vS is derived on-device from vT transposes)
    # blobA: [w1(512) | qT(512)]   blobB: [w2(512) | vT(1024) | ampsc | scN]
    BLOBA = 512 + 512 + 128
    BLOBB = 512 + 1024 + NSLOT * 2 + 2 * len(KONLY)
    blobA_in = nc.dram_tensor("blobA", [128, BLOBA], F16, kind="ExternalInput")
    blobB_in = nc.dram_tensor("blobB", [128, BLOBB], F16, kind="ExternalInput")
    mrow_in = nc.dram_tensor("mrow", [1, B, S], F32, kind="ExternalInput")
    ctx_out = nc.dram_tensor("context", [B, T, D], F32, kind="ExternalOutput")
    attn_out = nc.dram_tensor("attn", [B, T, S], F32, kind="ExternalOutput")


    with tile.TileContext(nc) as tc, ExitStack() as ctx:
        sg = ctx.enter_context(tc.tile_pool(name="sg", bufs=1))
        p_qu = ctx.enter_context(tc.tile_pool(name="p_qu", bufs=1, space="PSUM"))
        p_ku = ctx.enter_context(tc.tile_pool(name="p_ku", bufs=1, space="PSUM"))
        p_sc = ctx.enter_context(tc.tile_pool(name="p_sc", bufs=1, space="PSUM"))
        p_rc = ctx.enter_context(tc.tile_pool(name="p_rc", bufs=1, space="PSUM"))
        p_ct = ctx.enter_context(tc.tile_pool(name="p_ct", bufs=1, space="PSUM"))
        p_tp = ctx.enter_context(tc.tile_pool(name="p_tp", bufs=1, space="PSUM"))

        # ---- input DMAs (two blobs ordered by need + tiny ones)
        blobA = sg.tile([128, BLOBA], F16)
        nc.sync.dma_start(out=blobA, in_=blobA_in[:, :])
        blobB = sg.tile([128, BLOBB], F16)
        nc.gpsimd.dma_start(out=blobB, in_=blobB_in[:, :])
        w1 = blobA[:, 0:512].rearrange("p (j u) -> p j u", j=2)
        qT = blobA[:, 512:1024].rearrange("p (j b t) -> p j b t", j=2, b=B)
        w2 = blobB[:, 0:512].rearrange("p (j u) -> p j u", j=2)
        vT = blobB[:, 512:1536].rearrange("p (j b s) -> p j b s", j=2, b=B)
        ampsc = blobB[:, 1536:1536 + NSLOT * 2].rearrange(
            "p (f u) -> p f u", f=NSLOT)
        scN = blobB[:, 1536 + NSLOT * 2:BLOBB].rearrange(
            "p (u k) -> p u k", u=2)
        id16 = blobA[:, 1024:1152]
        ones16 = sg.tile([1, 128], F16)
        nc.vector.memset(ones16, 1.0)
        mrow = sg.tile([1, B, S], F32)
        nc.sync.dma_start(out=mrow, in_=mrow_in[:, :, :])

        sc_b = [p_sc.tile([128, S], F32, tag=f"scores{b}", name=f"sc{b}")
                for b in range(B)]

        # PE clock warm-up during input DMA
        wjunk = sg.tile([128, 512], F16)
        nc.vector.memset(wjunk, 0.0)
        for w in range(4):
            nc.tensor.matmul(
                sc_b[w % B],
                lhsT=wjunk[:, 0:128], rhs=wjunk[:, 0:256],
                start=True, stop=True,
            )

        pibias = sg.tile([128, 1], F32)
        nc.vector.memset(pibias, np.pi / 2)

        # ---- preamble: qU[u,(b,t)] = W1^T q^T ; kU[u,(b,s)] = W2^T v^T
        qU = p_qu.tile([128, B, 2, T], F32, tag="qU")
        for b in range(B):
            for ub in range(2):
                for j in range(2):
                    nc.tensor.matmul(
                        qU[:, b, ub, :],
                        lhsT=w1[:, j, ub * 128:(ub + 1) * 128],
                        rhs=qT[:, j, b, :],
                        start=(j == 0), stop=(j == 1),
                    )
        kU = p_ku.tile([128, B, 2, S], F32, tag="kU")
        for b in range(B):
            for ub in range(2):
                for j in range(2):
                    nc.tensor.matmul(
                        kU[:, b, ub, :],
                        lhsT=w2[:, j, ub * 128:(ub + 1) * 128],
                        rhs=vT[:, j, b, :],
                        start=(j == 0), stop=(j == 1),
                    )

        # vS[s-part, sblk, b, d] from vT via PE transposes; copies on GpSimd
        vS = sg.tile([128, 2, B, D], F16)
        for b in range(B):
            for sb in range(2):
                for j in range(2):
                    tp = p_tp.tile([128, 128], F16, tag="tp", name=f"tpv{b}{sb}{j}")
                    nc.tensor.transpose(
                        tp, vT[:, j, b, sb * 128:(sb + 1) * 128], id16)
                    nc.vector.tensor_copy(
                        out=vS[:, sb, b, j * 128:(j + 1) * 128], in_=tp)

        qstack = sg.tile([128, NSLOT, B, 2, T], F16)
        kstack = sg.tile([128, NSLOT, B, 2, S], F16)

        # base sinusoids on ScalarE (q side first so its ladder starts early)
        def sins(stack, src):
            nc.scalar.activation(out=stack[:, 0], in_=src, func=AF.Sin,
                                 scale=F1)
            nc.scalar.activation(out=stack[:, 1], in_=src, func=AF.Sin,
                                 scale=F1, bias=pibias)
            nc.scalar.activation(out=stack[:, 2], in_=src, func=AF.Sin,
                                 scale=F2)
            nc.scalar.activation(out=stack[:, 3], in_=src, func=AF.Sin,
                                 scale=F2, bias=pibias)
            nc.scalar.activation(out=stack[:, 4], in_=src, func=AF.Sin,
                                 scale=2 * F2)

        sins(kstack, kU[:, :, :, :])
        sins(qstack, qU[:, :, :, :])

        # doubling ladder: (sq_src, c_dst) then s_dst = 2 * s_src * c_dst
        def ladder(stack, scrtag, width, sq_engines):
            for (src, cdst, sprev, sdst), eng in zip(
                    ((2, 5, 4, 6), (4, 7, 6, 8), (6, 9, None, None)),
                    sq_engines):
                scr = sg.tile([128, B, 2, width], F16, tag=f"{scrtag}{src}")
                if eng == "act":
                    nc.scalar.activation(out=scr, in_=stack[:, src],
                                         func=AF.Square)
                elif eng == "gp":
                    nc.gpsimd.tensor_tensor(out=scr, in0=stack[:, src],
                                            in1=stack[:, src], op=ALU.mult)
                else:
                    nc.vector.tensor_tensor(out=scr, in0=stack[:, src],
                                            in1=stack[:, src], op=ALU.mult)
                nc.vector.tensor_scalar(out=stack[:, cdst], in0=scr,
                                        scalar1=-2.0, scalar2=1.0,
                                        op0=ALU.mult, op1=ALU.add)
                if sdst is not None:
                    nc.vector.scalar_tensor_tensor(
                        out=stack[:, sdst], in0=stack[:, sprev], scalar=2.0,
                        in1=stack[:, cdst], op0=ALU.mult, op1=ALU.mult)

        ladder(qstack, "sq", T, ("gp", "gp", "gp"))

        # fold A_r * scale_u into the q-side stack (fp16, per u-block) —
        # emitted before the k ladder so PE rank matmuls can start on the
        # Act-direct k slots while DVE finishes the k ladder
        qsc = sg.tile([128, NSLOT, B, 2, T], F16)

        def fold(ub, lo, hi):
            col = ampsc[:, lo:hi, ub]
            amp_ap = bass.AP(
                tensor=col.tensor, offset=col.offset,
                ap=[list(col.ap[0]), list(col.ap[1]), [0, B], [0, T]],
            )
            nc.vector.tensor_tensor(
                out=qsc[:, lo:hi, :, ub, :], in0=qstack[:, lo:hi, :, ub, :],
                in1=amp_ap, op=ALU.mult)

        # direct slots fold first (unblocks the direct-k rank matmuls),
        # ladder slots fold after the k-ladder's DVE ops
        fold(0, 0, 5)
        fold(1, 0, 5)
        ladder(kstack, "sk", S, ("act", "act", "dve"))
        fold(0, 5, NSLOT)
        fold(1, 5, NSLOT)
        # dummy exp hoists the Exp table load into the matmul phase; reads
        # a k-ladder output so the scheduler keeps it after the Sin/Square ops
        dummy = sg.tile([128, 1], F32)
        nc.scalar.activation(out=dummy, in_=kstack[:, 9, 0, 0, 0:1],
                             func=AF.Exp)

        # ---- k-only ranks -> bias row (PSUM row 0), + mask row
        rows = p_rc.tile([128, B, S], F32, tag="rows")
        for b in range(B):
            i = 0
            n = 2 * len(KONLY)
            for ki, (ks, _a) in enumerate(KONLY):
                for ub in range(2):
                    nc.tensor.matmul(
                        rows[0:1, b, :],
                        lhsT=scN[:, ub, ki:ki + 1],
                        rhs=kstack[:, ks, b, ub, :],
                        start=(i == 0), stop=(i == n - 1),
                    )
                    i += 1
        brow0 = sg.tile([1, B, S], F32)
        brow = sg.tile([1, B, S], F16)

        # ---- per-batch pipeline: scores -> softmax -> context, so batch 1
        # matmuls overlap batch 0 softmax/epilogue
        ORDER = [(0, 1), (1, 2), (5, 4), (7, 6), (6, 7), (2, 7), (9, 8),
                 (8, 9)]
        e = sg.tile([128, B, S], F32)
        esum = sg.tile([128, B, 1], F32)
        inv = sg.tile([128, B, 1], F32)
        attn_f = sg.tile([128, B, S], F32)
        e16 = sg.tile([128, B, S], F16)
        attnT = sg.tile([128, 2, B, T], F16)
        ctxp = p_ct.tile([128, B, D], F32, tag="ctxp")
        ctx_f = sg.tile([128, B, D], F32)
        for b in range(B):
            nc.scalar.activation(out=brow0[:, b, :], in_=rows[0:1, b, :],
                                 func=AF.Copy)
            nc.gpsimd.tensor_tensor(out=brow[:, b, :], in0=brow0[:, b, :],
                                    in1=mrow[:, b, :], op=ALU.add)
            i = 0
            for ub in range(2):
                for (qs, ks) in ORDER:
                    nc.tensor.matmul(
                        sc_b[b],
                        lhsT=qsc[:, qs, b, ub, :],
                        rhs=kstack[:, ks, b, ub, :],
                        start=(i == 0), stop=False,
                    )
                    i += 1
            nc.tensor.matmul(
                sc_b[b], lhsT=ones16, rhs=brow[0:1, b, :],
                start=False, stop=True,
            )
            # softmax for this batch (overlaps next batch's matmuls)
            nc.scalar.activation(out=e[:, b, :], in_=sc_b[b],
                                 func=AF.Exp, accum_out=esum[:, b, :])
            nc.vector.reciprocal(out=inv[:, b, :], in_=esum[:, b, :])
            nc.vector.tensor_scalar_mul(out=attn_f[:, b, :], in0=e[:, b, :],
                                        scalar1=inv[:, b, :])
            nc.sync.dma_start(out=attn_out[b], in_=attn_f[:, b, :])
            nc.vector.tensor_scalar_mul(out=e16[:, b, :], in0=e[:, b, :],
                                        scalar1=2.0 ** -6)
            for sb in range(2):
                tp = p_tp.tile([128, 128], F16, tag="tp")
                nc.tensor.transpose(tp, e16[:, b, sb * 128:(sb + 1) * 128],
                                    id16)
                nc.vector.tensor_copy(out=attnT[:, sb, b, :], in_=tp)
            for sb in range(2):
                nc.tensor.matmul(
                    ctxp[:, b, :], lhsT=attnT[:, sb, b, :],
                    rhs=vS[:, sb, b, :],
                    start=(sb == 0), stop=(sb == 1),
                )
            # ctx = ctxp * inv * 64 in one tensor_scalar (two scalar ops)
            nc.vector.tensor_scalar(out=ctx_f[:, b, :], in0=ctxp[:, b, :],
                                    scalar1=inv[:, b, :], scalar2=64.0,
                                    op0=ALU.mult, op1=ALU.mult)
            nc.sync.dma_start(out=ctx_out[b], in_=ctx_f[:, b, :])

    nc.compile()
    return nc


_BUILT: bass.Bass | None = None


def _get_built() -> bass.Bass:
    global _BUILT
    if _BUILT is None:
        _BUILT = build_bass()
    return _BUILT


def make_in_maps(query, value, mask, W1, W2, scale):
    q16 = np.asarray(query, dtype=np.float16)
    v16 = np.asarray(value, dtype=np.float16)
    m = np.asarray(mask).astype(np.float32)
    w1 = np.asarray(W1, dtype=np.float16)
    w2 = np.asarray(W2, dtype=np.float16)
    sc = np.asarray(scale, dtype=np.float32)

    w1h = np.ascontiguousarray(w1.reshape(2, 128, U).transpose(1, 0, 2))
    w2h = np.ascontiguousarray(w2.reshape(2, 128, U).transpose(1, 0, 2))
    scT = sc.reshape(2, 128).T                       # (128, 2) by u-block
    ampsc = np.ascontiguousarray(
        (AMPQ[None, :, None] * scT[:, None, :]).astype(np.float16))
    scn = np.stack([a * sc for (_ks, a) in KONLY], axis=1)  # (256, nk)
    scN = np.ascontiguousarray(
        scn.reshape(2, 128, len(KONLY)).transpose(1, 0, 2).astype(np.float16))

    in_maps = []
    for c in range(N_CORES):
        sl = slice(B * c, B * (c + 1))
        q = q16[sl]                      # (B, T, D)
        v = v16[sl]                      # (B, S, D)
        qTh = np.ascontiguousarray(
            q.reshape(B, T, 2, 128).transpose(3, 2, 0, 1))
        vTh = np.ascontiguousarray(
            v.reshape(B, S, 2, 128).transpose(3, 2, 0, 1))
        mrow = np.ascontiguousarray(
            ((m[sl] - 1.0) * 30000.0)[None, :, :].astype(np.float32))
        blobA = np.ascontiguousarray(np.concatenate(
            [a.reshape(128, -1) for a in
             (w1h, qTh, np.eye(128, dtype=np.float16))], axis=1))
        blobB = np.ascontiguousarray(np.concatenate(
            [a.reshape(128, -1) for a in (w2h, vTh, ampsc, scN)], axis=1))
        in_maps.append({"blobA": blobA, "blobB": blobB, "mrow": mrow})
    return in_maps


def run(query, value, mask, W1, W2, scale, trace=False, **trace_kwargs):
    nc = _get_built()
    in_maps = make_in_maps(query, value, mask, W1, W2, scale)
    res = run_bass_kernel_spmd(
        nc, in_maps, core_ids=list(range(N_CORES)), trace=trace, **trace_kwargs
    )
    context = np.concatenate([r["context"] for r in res.results], axis=0)
    attn = np.concatenate([r["attn"] for r in res.results], axis=0)
    return (context, attn), res


def kernel(query, value, mask, W1, W2, scale):
    (context, attn), _ = run(query, value, mask, W1, W2, scale, trace=False)
    return context, attn


if __name__ == "__main__":
    build_bass()
    print("build OK")


# revision 36
# speedup vs baseline: 1.0626x; 1.0579x over previous
"""Bahdanau (additive) attention Trainium2 kernel — factorized-score version.

Full-input contract: kernel(**inputs) takes the unsharded inputs
(query [16,128,256], value [16,256,256], mask [16,256], W1 [256,256],
W2 [256,256], scale [256]) and returns (context, attn_weights), both
[16,128,256] float32, matching the jax reference.

Sharding: data-parallel over batch -> 8 NeuronCores x 2 batches each.

Algorithm (replaces elementwise tanh over t*s*u = 16.8M elems/core):
  tanh(q+k) ~ g(q) + sum_r A_r F_r(q) G_r(k) with sinusoid factors at
  log-spaced frequencies {0.16, 0.28, 0.56, 1.12, 2.24}:
    slots 0-4: sin/cos at 0.16, 0.28 and sin at 0.56 — direct ScalarE Sin
               (args stay inside the +-3.3 rad domain of the HW table)
    slots 5-9: cos 0.56, sin/cos 1.12, sin/cos 2.24 via fp16 doubling
               identities (c2m = 1-2 s_m^2, s2m = 2 s_m c_m); squares on
               GpSimd (q side) / ScalarE Square (k side), rest on DVE
  g(q) is dropped (row-constant cancels in softmax). 9 product ranks +
  4 k-only ranks + the mask row fold into PE matmuls accumulating
  scores[t,s] in PSUM; A_r*scale_u folds into one fp16 multiply per
  u-block on the q-side stack (broadcast table, stride-0 AP).
  softmax: Exp on ScalarE with fused row sums; attn out via Copy(scale).
  context = attn @ value with PE transposes of e (scaled 2^-6, fp16).

Fit (vs f64 reference, incl. fp16 emulation): rel err ctx 9.6e-3,
attn 1.03e-2 (tolerance 2e-2).
"""

import sys

if "/opt/trn_rl_repo" not in sys.path:
    sys.path.insert(0, "/opt/trn_rl_repo")

from contextlib import ExitStack

import numpy as np

import concourse.bacc as bacc
import concourse.bass as bass
import concourse.tile as tile
from concourse import mybir
from concourse.bass_utils import run_bass_kernel_spmd

F32 = mybir.dt.float32
F16 = mybir.dt.float16
AF = mybir.ActivationFunctionType
ALU = mybir.AluOpType

N_CORES = 8
B = 2          # batches per core
T = 128        # query rows
S = 256        # kv rows
D = 256        # d_model
U = 256        # units
NSLOT = 10

F1 = 0.16
F2 = 0.28

# slots: 0:s(f1) 1:c(f1) 2:s(f2) 3:c(f2) 4:s(2f2)
#        5:c(2f2) 6:s(4f2) 7:c(4f2) 8:s(8f2) 9:c(8f2)
# (q-slot, k-slot, A); q-slots injective so A_r*scale_u folds per q-slot
RANKS = [
    (0, 1, 9.076809),
    (1, 2, 23.773289),
    (5, 4, -1.240546),
    (7, 6, 0.421618),
    (6, 7, 0.344957),
    (2, 7, -0.017345),
    (9, 8, 0.072694),
    (8, 9, 0.077664),
]
KONLY = [(0, -13.445086), (2, -12.153659), (4, -0.148668)]

AMPQ = np.zeros(NSLOT, dtype=np.float32)
for _qs, _ks, _a in RANKS:
    AMPQ[_qs] = _a


def build_bass() -> bass.Bass:
    nc = bacc.Bacc("TRN2", target_bir_lowering=False, debug=False)

    # fused fp16 input blobs (vS is derived on-device from vT transposes)
    # blobA: [w1(512) | qT(512)]   blobB: [w2(512) | vT(1024) | ampsc | scN]
    BLOBA = 512 + 512 + 128
    BLOBB = 512 + 1024 + NSLOT * 2 + 2 * len(KONLY)
    blobA_in = nc.dram_tensor("blobA", [128, BLOBA], F16, kind="ExternalInput")
    blobB_in = nc.dram_tensor("blobB", [128, BLOBB], F16, kind="ExternalInput")
    ampf_in = nc.dram_tensor("ampf", [128, NSLOT, 2], F32, kind="ExternalInput")
    mrow_in = nc.dram_tensor("mrow", [1, B, S], F32, kind="ExternalInput")
    ctx_out = nc.dram_tensor("context", [B, T, D], F32, kind="ExternalOutput")
    attn_out = nc.dram_tensor("attn", [B, T, S], F32, kind="ExternalOutput")


    with tile.TileContext(nc) as tc, ExitStack() as ctx:
        sg = ctx.enter_context(tc.tile_pool(name="sg", bufs=1))
        p_qu = ctx.enter_context(tc.tile_pool(name="p_qu", bufs=1, space="PSUM"))
        p_ku = ctx.enter_context(tc.tile_pool(name="p_ku", bufs=1, space="PSUM"))
        p_sc = ctx.enter_context(tc.tile_pool(name="p_sc", bufs=1, space="PSUM"))
        p_rc = ctx.enter_context(tc.tile_pool(name="p_rc", bufs=1, space="PSUM"))
        p_ct = ctx.enter_context(tc.tile_pool(name="p_ct", bufs=1, space="PSUM"))
        p_tp = ctx.enter_context(tc.tile_pool(name="p_tp", bufs=1, space="PSUM"))

        # ---- input DMAs (two blobs ordered by need + tiny ones)
        blobA = sg.tile([128, BLOBA], F16)
        nc.sync.dma_start(out=blobA, in_=blobA_in[:, :])
        blobB = sg.tile([128, BLOBB], F16)
        nc.gpsimd.dma_start(out=blobB, in_=blobB_in[:, :])
        w1 = blobA[:, 0:512].rearrange("p (j u) -> p j u", j=2)
        qT = blobA[:, 512:1024].rearrange("p (j b t) -> p j b t", j=2, b=B)
        w2 = blobB[:, 0:512].rearrange("p (j u) -> p j u", j=2)
        vT = blobB[:, 512:1536].rearrange("p (j b s) -> p j b s", j=2, b=B)
        ampsc = blobB[:, 1536:1536 + NSLOT * 2].rearrange(
            "p (f u) -> p f u", f=NSLOT)
        scN = blobB[:, 1536 + NSLOT * 2:BLOBB].rearrange(
            "p (u k) -> p u k", u=2)
        id16 = blobA[:, 1024:1152]
        ones16 = sg.tile([1, 128], F16)
        nc.vector.memset(ones16, 1.0)
        mrow = sg.tile([1, B, S], F32)
        nc.sync.dma_start(out=mrow, in_=mrow_in[:, :, :])
        ampf = sg.tile([128, NSLOT, 2], F32)
        nc.sync.dma_start(out=ampf, in_=ampf_in[:, :, :])

        sc_b = [p_sc.tile([128, S], F32, tag=f"scores{b}", name=f"sc{b}")
                for b in range(B)]

        # PE clock warm-up during input DMA
        wjunk = sg.tile([128, 512], F16)
        nc.vector.memset(wjunk, 0.0)
        for w in range(4):
            nc.tensor.matmul(
                sc_b[w % B],
                lhsT=wjunk[:, 0:128], rhs=wjunk[:, 0:256],
                start=True, stop=True,
            )

        pibias = sg.tile([128, 1], F32)
        nc.vector.memset(pibias, np.pi / 2)

        # ---- preamble: qU[u,(b,t)] = W1^T q^T ; kU[u,(b,s)] = W2^T v^T
        qU = p_qu.tile([128, B, 2, T], F32, tag="qU")
        for b in range(B):
            for ub in range(2):
                for j in range(2):
                    nc.tensor.matmul(
                        qU[:, b, ub, :],
                        lhsT=w1[:, j, ub * 128:(ub + 1) * 128],
                        rhs=qT[:, j, b, :],
                        start=(j == 0), stop=(j == 1),
                    )
        kU = p_ku.tile([128, B, 2, S], F32, tag="kU")
        for b in range(B):
            for ub in range(2):
                for j in range(2):
                    nc.tensor.matmul(
                        kU[:, b, ub, :],
                        lhsT=w2[:, j, ub * 128:(ub + 1) * 128],
                        rhs=vT[:, j, b, :],
                        start=(j == 0), stop=(j == 1),
                    )

        # vS[s-part, sblk, b, d] from vT via PE transposes; copies on GpSimd
        vS = sg.tile([128, 2, B, D], F16)
        for b in range(B):
            for sb in range(2):
                for j in range(2):
                    tp = p_tp.tile([128, 128], F16, tag="tp", name=f"tpv{b}{sb}{j}")
                    nc.tensor.transpose(
                        tp, vT[:, j, b, sb * 128:(sb + 1) * 128], id16)
                    nc.vector.tensor_copy(
                        out=vS[:, sb, b, j * 128:(j + 1) * 128], in_=tp)

        qstack = sg.tile([128, NSLOT, B, 2, T], F16)
        kstack = sg.tile([128, NSLOT, B, 2, S], F16)

        # base sinusoids on ScalarE (q side first so its ladder starts early)
        def sins(stack, src):
            nc.scalar.activation(out=stack[:, 0], in_=src, func=AF.Sin,
                                 scale=F1)
            nc.scalar.activation(out=stack[:, 1], in_=src, func=AF.Sin,
                                 scale=F1, bias=pibias)
            nc.scalar.activation(out=stack[:, 2], in_=src, func=AF.Sin,
                                 scale=F2)
            nc.scalar.activation(out=stack[:, 3], in_=src, func=AF.Sin,
                                 scale=F2, bias=pibias)
            nc.scalar.activation(out=stack[:, 4], in_=src, func=AF.Sin,
                                 scale=2 * F2)

        sins(kstack, kU[:, :, :, :])
        sins(qstack, qU[:, :, :, :])

        # doubling ladder: (sq_src, c_dst) then s_dst = 2 * s_src * c_dst
        def ladder(stack, scrtag, width, sq_engines):
            for (src, cdst, sprev, sdst), eng in zip(
                    ((2, 5, 4, 6), (4, 7, 6, 8), (6, 9, None, None)),
                    sq_engines):
                scr = sg.tile([128, B, 2, width], F16, tag=f"{scrtag}{src}")
                if eng == "act":
                    nc.scalar.activation(out=scr, in_=stack[:, src],
                                         func=AF.Square)
                elif eng == "gp":
                    nc.gpsimd.tensor_tensor(out=scr, in0=stack[:, src],
                                            in1=stack[:, src], op=ALU.mult)
                else:
                    nc.vector.tensor_tensor(out=scr, in0=stack[:, src],
                                            in1=stack[:, src], op=ALU.mult)
                nc.vector.tensor_scalar(out=stack[:, cdst], in0=scr,
                                        scalar1=-2.0, scalar2=1.0,
                                        op0=ALU.mult, op1=ALU.add)
                if sdst is not None:
                    nc.vector.scalar_tensor_tensor(
                        out=stack[:, sdst], in0=stack[:, sprev], scalar=2.0,
                        in1=stack[:, cdst], op0=ALU.mult, op1=ALU.mult)

        ladder(qstack, "sq", T, ("gp", "gp", "gp"))

        # fold A_r * scale_u into the q-side stack (fp16, per u-block) —
        # emitted before the k ladder so PE rank matmuls can start on the
        # Act-direct k slots while DVE finishes the k ladder
        qsc = sg.tile([128, NSLOT, B, 2, T], F16)

        def fold(ub, lo, hi):
            col = ampsc[:, lo:hi, ub]
            amp_ap = bass.AP(
                tensor=col.tensor, offset=col.offset,
                ap=[list(col.ap[0]), list(col.ap[1]), [0, B], [0, T]],
            )
            nc.vector.tensor_tensor(
                out=qsc[:, lo:hi, :, ub, :], in0=qstack[:, lo:hi, :, ub, :],
                in1=amp_ap, op=ALU.mult)

        # direct slots fold first (unblocks the direct-k rank matmuls),
        # ladder slots fold after the k-ladder's DVE ops
        fold(0, 0, 5)
        fold(1, 0, 5)
        ladder(kstack, "sk", S, ("act", "act", "dve"))
        # ladder-slot folds on the otherwise-idle ScalarE (Copy with
        # per-partition f32 scale = A_f*scale_u); frees ~2.8us of DVE
        for f in range(5, NSLOT):
            for ub in range(2):
                nc.scalar.activation(out=qsc[:, f, :, ub, :],
                                     in_=qstack[:, f, :, ub, :],
                                     func=AF.Copy, scale=ampf[:, f, ub:ub + 1])
        # dummy exp hoists the Exp table load into the matmul phase; reads
        # a k-ladder output so the scheduler keeps it after the Sin/Square ops
        dummy = sg.tile([128, 1], F32)
        nc.scalar.activation(out=dummy, in_=kstack[:, 9, 0, 0, 0:1],
                             func=AF.Exp)

        # ---- k-only ranks -> bias row (PSUM row 0), + mask row
        rows = p_rc.tile([128, B, S], F32, tag="rows")
        for b in range(B):
            i = 0
            n = 2 * len(KONLY)
            for ki, (ks, _a) in enumerate(KONLY):
                for ub in range(2):
                    nc.tensor.matmul(
                        rows[0:1, b, :],
                        lhsT=scN[:, ub, ki:ki + 1],
                        rhs=kstack[:, ks, b, ub, :],
                        start=(i == 0), stop=(i == n - 1),
                    )
                    i += 1
        brow0 = sg.tile([1, B, S], F32)
        brow = sg.tile([1, B, S], F16)

        # ---- per-batch pipeline: scores -> softmax -> context, so batch 1
        # matmuls overlap batch 0 softmax/epilogue
        ORDER = [(0, 1), (1, 2), (5, 4), (7, 6), (6, 7), (2, 7), (9, 8),
                 (8, 9)]
        e = sg.tile([128, B, S], F32)
        esum = sg.tile([128, B, 1], F32)
        inv = sg.tile([128, B, 1], F32)
        attn_f = sg.tile([128, B, S], F32)
        e16 = sg.tile([128, B, S], F16)
        attnT = sg.tile([128, 2, B, T], F16)
        ctxp = p_ct.tile([128, B, D], F32, tag="ctxp")
        ctx_f = sg.tile([128, B, D], F32)
        for b in range(B):
            nc.scalar.activation(out=brow0[:, b, :], in_=rows[0:1, b, :],
                                 func=AF.Copy)
            nc.gpsimd.tensor_tensor(out=brow[:, b, :], in0=brow0[:, b, :],
                                    in1=mrow[:, b, :], op=ALU.add)
            i = 0
            for ub in range(2):
                for (qs, ks) in ORDER:
                    nc.tensor.matmul(
                        sc_b[b],
                        lhsT=qsc[:, qs, b, ub, :],
                        rhs=kstack[:, ks, b, ub, :],
                        start=(i == 0), stop=False,
                    )
                    i += 1
            nc.tensor.matmul(
                sc_b[b], lhsT=ones16, rhs=brow[0:1, b, :],
                start=False, stop=True,
            )
            # softmax for this batch (overlaps next batch's matmuls)
            nc.scalar.activation(out=e[:, b, :], in_=sc_b[b],
                                 func=AF.Exp, accum_out=esum[:, b, :])
            nc.vector.reciprocal(out=inv[:, b, :], in_=esum[:, b, :])
            nc.vector.tensor_scalar_mul(out=attn_f[:, b, :], in0=e[:, b, :],
                                        scalar1=inv[:, b, :])
            nc.sync.dma_start(out=attn_out[b], in_=attn_f[:, b, :])
            nc.vector.tensor_scalar_mul(out=e16[:, b, :], in0=e[:, b, :],
                                        scalar1=2.0 ** -6)
            for sb in range(2):
                tp = p_tp.tile([128, 128], F16, tag="tp")
                nc.tensor.transpose(tp, e16[:, b, sb * 128:(sb + 1) * 128],
                                    id16)
                nc.vector.tensor_copy(out=attnT[:, sb, b, :], in_=tp)
            for sb in range(2):
                nc.tensor.matmul(
                    ctxp[:, b, :], lhsT=attnT[:, sb, b, :],
                    rhs=vS[:, sb, b, :],
                    start=(sb == 0), stop=(sb == 1),
                )
            # ctx = ctxp * inv * 64 in one tensor_scalar (two scalar ops)
            nc.vector.tensor_scalar(out=ctx_f[:, b, :], in0=ctxp[:, b, :],
                                    scalar1=inv[:, b, :], scalar2=64.0,
                                    op0=ALU.mult, op1=ALU.mult)
            nc.sync.dma_start(out=ctx_out[b], in_=ctx_f[:, b, :])

    nc.compile()
    return nc


_BUILT: bass.Bass | None = None


def _get_built() -> bass.Bass:
    global _BUILT
    if _BUILT is None:
        _BUILT = build_bass()
    return _BUILT


def make_in_maps(query, value, mask, W1, W2, scale):
    q16 = np.asarray(query, dtype=np.float16)
    v16 = np.asarray(value, dtype=np.float16)
    m = np.asarray(mask).astype(np.float32)
    w1 = np.asarray(W1, dtype=np.float16)
    w2 = np.asarray(W2, dtype=np.float16)
    sc = np.asarray(scale, dtype=np.float32)

    w1h = np.ascontiguousarray(w1.reshape(2, 128, U).transpose(1, 0, 2))
    w2h = np.ascontiguousarray(w2.reshape(2, 128, U).transpose(1, 0, 2))
    scT = sc.reshape(2, 128).T                       # (128, 2) by u-block
    ampsc = np.ascontiguousarray(
        (AMPQ[None, :, None] * scT[:, None, :]).astype(np.float16))
    scn = np.stack([a * sc for (_ks, a) in KONLY], axis=1)  # (256, nk)
    scN = np.ascontiguousarray(
        scn.reshape(2, 128, len(KONLY)).transpose(1, 0, 2).astype(np.float16))

    in_maps = []
    for c in range(N_CORES):
        sl = slice(B * c, B * (c + 1))
        q = q16[sl]                      # (B, T, D)
        v = v16[sl]                      # (B, S, D)
        qTh = np.ascontiguousarray(
            q.reshape(B, T, 2, 128).transpose(3, 2, 0, 1))
        vTh = np.ascontiguousarray(
            v.reshape(B, S, 2, 128).transpose(3, 2, 0, 1))
        mrow = np.ascontiguousarray(
            ((m[sl] - 1.0) * 30000.0)[None, :, :].astype(np.float32))
        blobA = np.ascontiguousarray(np.concatenate(
            [a.reshape(128, -1) for a in
             (w1h, qTh, np.eye(128, dtype=np.float16))], axis=1))
        ampf = np.ascontiguousarray(
            (AMPQ[None, :, None] * scT[:, None, :]).astype(np.float32))
        blobB = np.ascontiguousarray(np.concatenate(
            [a.reshape(128, -1) for a in (w2h, vTh, ampsc, scN)], axis=1))
        in_maps.append({"blobA": blobA, "blobB": blobB, "mrow": mrow,
                        "ampf": ampf})
    return in_maps


def run(query, value, mask, W1, W2, scale, trace=False, **trace_kwargs):
    nc = _get_built()
    in_maps = make_in_maps(query, value, mask, W1, W2, scale)
    res = run_bass_kernel_spmd(
        nc, in_maps, core_ids=list(range(N_CORES)), trace=trace, **trace_kwargs
    )
    context = np.concatenate([r["context"] for r in res.results], axis=0)
    attn = np.concatenate([r["attn"] for r in res.results], axis=0)
    return (context, attn), res


def kernel(query, value, mask, W1, W2, scale):
    (context, attn), _ = run(query, value, mask, W1, W2, scale, trace=False)
    return context, attn


if __name__ == "__main__":
    build_bass()
    print("build OK")
